# revision 35
# baseline (speedup 1.0000x reference)
"""N-gram embedding lookup kernel for Trainium2 (8 NeuronCores, Bass/Tile).

Problem: for each token x[b,s] (vocab 50000), gather precomputed n-gram
hash ids for orders 1..3 (12+11+10 slots), gather embedding rows from
three tables (1001/10001/50001 x 256 fp32), masked-mean each order,
concat to 768 dims; tokens x<4 take tab0[x] instead.

Design (count-matmul for orders 1/2, bulk-gather ucode for order 3;
316.8us baseline -> 61.5us, rel err 1.655e-3 unchanged):
 - KEY STRUCTURE: order-1 grams are single characters => <=26 distinct
   hash ids ever occur; order-2 grams are character pairs => <=676
   distinct ids. So sum_s tab[id_s] = counts @ T where counts is a tiny
   per-word histogram (pure index data, built on host) and T is the
   table restricted to the occurring ids. Orders 1+2 become one K=32
   and one K=768 PE matmul per 128-word group - no gather descriptors
   at all. Order-3 (26^3 = 17576 possible ids) stays a real gather.
 - host: dedup x to unique words, sort by word length, deal round-robin
   to 8 cores => each 128-word group has near-uniform slot counts.
 - host: per core, compact the order-3 rows actually referenced into a
   per-core table (row-sharded vocab-parallel, data-aware): tabC[0]=0,
   tabC[1+j] = j-th unique row; ~9k rows, int16-indexable.
 - chip order 3: per group, gpsimd.dma_gather (InstDMAGatherAnt) bulk-
   gathers slot rows [word-partition, slot-col, 256] bf16 in <=8-column
   (1024-index) chunks round-robin over the 4 SWDGE queues (~4ns/row;
   >1024 indexes per instruction overflows the ucode descriptor ring).
 - chip orders 1+2: PE matmuls of host-built count matrices against
   SBUF-resident letter/bigram tables, accumulated in PSUM.
 - DVE reduces the order-3 slot columns (bf16 -> f32); ACT applies the
   1/cnt scales and drains PSUM; SP stores [128,768] per group.
"""
import numpy as np
import ml_dtypes
from contextlib import ExitStack

from concourse import bacc, bass, mybir, tile
from concourse.bass_utils import run_bass_kernel_spmd

BF16 = ml_dtypes.bfloat16

NCORES = 8
B, S = 8, 2048
P = 128
EMB = 256
V = 50000
NQ = 4                        # SWDGE queues (ucode max)
CHUNK = 8                     # slot-columns per dma_gather (1024 descs max)
ROT = 5                       # slab rotation depth
PSROT = 4                     # psum rotation depth per order
TABC_ROWS = 12288             # per-core compacted order-3 table rows
NROWS = (1005, 10005, 50005)  # rows in tab{1,2,3}z incl. 4 special rows
K1 = 32                       # order-1 id space (<=26 letters + 4 specials)
K2 = 768                      # order-2 id space (<=676 bigrams + 4 specials)
NB2 = K2 // P


def _build(profile, unroll=1, chunk=CHUNK, rot=ROT, parts="gmts"):
    """profile: tuple per group of (L3,) max order-3 slot counts.
    parts: g=gathers m=matmuls t=tails s=stores (debug decomposition)."""
    G = len(profile)
    TPC = G * P
    i16, f32, bf16 = mybir.dt.int16, mybir.dt.float32, mybir.dt.bfloat16
    nc = bacc.Bacc("TRN2", target_bir_lowering=False, debug=False,
                   num_devices=NCORES, num_swdge_queues=NQ)

    cols_g = [p[0] for p in profile]
    W = sum(cols_g) * 8            # idx i16 elements per partition row

    d_tabc = nc.dram_tensor("tabc", [TABC_ROWS, EMB], bf16,
                            kind="ExternalInput")
    d_idx = nc.dram_tensor("idxs", [P, W], i16, kind="ExternalInput")
    d_rcp = nc.dram_tensor("rcp", [P, G * 3], f32, kind="ExternalInput")
    d_t1 = nc.dram_tensor("t1r", [K1, EMB], bf16, kind="ExternalInput")
    d_t2 = nc.dram_tensor("t2r", [P, NB2 * EMB], bf16, kind="ExternalInput")
    d_lc1 = nc.dram_tensor("lc1", [K1, G * P], bf16, kind="ExternalInput")
    d_lc2 = nc.dram_tensor("lc2", [P, NB2 * G * P], bf16,
                           kind="ExternalInput")
    d_out = nc.dram_tensor("out", [TPC, 768], f32, kind="ExternalOutput")

    qctr = [0]

    with ExitStack() as ctx:
        tc = ctx.enter_context(tile.TileContext(nc))
        pool = ctx.enter_context(tc.tile_pool(name="sbuf", bufs=1))
        psum = ctx.enter_context(
            tc.tile_pool(name="psum", bufs=1, space="PSUM"))

        t_idx = pool.tile([P, W], i16)
        t_rcp = pool.tile([P, G * 3], f32)
        t_out = pool.tile([P, G * 768], f32)
        t_t1 = pool.tile([P, EMB], bf16)
        t_t2 = pool.tile([P, NB2 * EMB], bf16)
        t_lc1 = pool.tile([P, G * P], bf16)
        t_lc2 = pool.tile([P, NB2 * G * P], bf16)
        colsmax = max(cols_g)
        slabs = [pool.tile([P, colsmax * EMB], bf16, name=f"slab{i}")
                 for i in range(rot)]
        ps1 = [psum.tile([P, EMB], f32, name=f"ps1_{i}")
               for i in range(PSROT)]
        ps2 = [psum.tile([P, EMB], f32, name=f"ps2_{i}")
               for i in range(PSROT)]

        # small tables: load once (weights-like)
        nc.sync.dma_start(out=t_t1[0:K1, :], in_=d_t1[:])
        nc.sync.dma_start(out=t_t2[:], in_=d_t2[:])

        if unroll > 1:
            # hardware loop for benchmarking: body is idempotent
            ctx.enter_context(tc.For_i(0, unroll))

        # split the idx load so group 0's gathers unblock immediately
        w0 = cols_g[0] * 8
        nc.sync.dma_start(out=t_idx[:, 0:w0], in_=d_idx[:, 0:w0])
        nc.sync.dma_start(out=t_idx[:, w0:], in_=d_idx[:, w0:])
        nc.sync.dma_start(out=t_rcp[:], in_=d_rcp[:])
        nc.scalar.dma_start(out=t_lc1[0:K1, :], in_=d_lc1[:])
        # per-chunk lc2 loads so group 0's matmuls unblock chunk by chunk
        for k in range(NB2):
            nc.scalar.dma_start(
                out=t_lc2[:, k * G * P:(k + 1) * G * P],
                in_=d_lc2[:, k * G * P:(k + 1) * G * P])

        idx_off = [0]

        def emit_gather(g):
            slab = slabs[g % rot]
            cols = cols_g[g]
            c0 = 0
            while c0 < cols:
                cc = min(chunk, cols - c0)
                n = cc * P
                nc.gpsimd.dma_gather(
                    out_ap=bass.AP(slab[:].tensor, c0 * EMB,
                                   [slab[:].ap[0], [EMB, cc], [1, EMB]]),
                    in_ap=d_tabc[:],
                    idxs_ap=t_idx[:, idx_off[0]:idx_off[0] + cc * 8],
                    num_idxs=n,
                    num_idxs_reg=n,
                    elem_size=EMB,
                    queue_num=qctr[0] % NQ,
                )
                qctr[0] += 1
                idx_off[0] += cc * 8
                c0 += cc

        def emit_matmuls(g):
            nc.tensor.matmul(
                ps1[g % PSROT][:],
                lhsT=t_lc1[0:K1, g * P:(g + 1) * P],
                rhs=t_t1[0:K1, :],
                start=True, stop=True)
            pB = ps2[g % PSROT]
            for k in range(NB2):
                nc.tensor.matmul(
                    pB[:],
                    lhsT=t_lc2[:, (k * G + g) * P:(k * G + g + 1) * P],
                    rhs=t_t2[:, k * EMB:(k + 1) * EMB],
                    start=(k == 0), stop=(k == NB2 - 1))

        def emit_tail(g):
            slab = slabs[g % rot]
            L3 = profile[g][0]
            nc.scalar.mul(t_out[:, g * 768:g * 768 + 256],
                          ps1[g % PSROT][:], t_rcp[:, g * 3:g * 3 + 1])
            nc.scalar.mul(t_out[:, g * 768 + 256:g * 768 + 512],
                          ps2[g % PSROT][:], t_rcp[:, g * 3 + 1:g * 3 + 2])
            out_ap = t_out[:, g * 768 + 512:g * 768 + 768]
            rcp_ap = t_rcp[:, g * 3 + 2:g * 3 + 3]
            if L3 == 1:
                nc.scalar.mul(out_ap, slab[:, 0:EMB], rcp_ap)
            else:
                nc.vector.tensor_reduce(
                    out=out_ap,
                    in_=bass.AP(slab[:].tensor, 0,
                                [slab[:].ap[0], [1, EMB], [EMB, L3]]),
                    axis=mybir.AxisListType.X,
                    op=mybir.AluOpType.add, opt_input=False)
                nc.scalar.mul(out_ap, out_ap, rcp_ap)
            if "s" in parts:
                nc.sync.dma_start(
                    out=bass.AP(d_out, g * P * 768, [[768, P], [1, 768]]),
                    in_=t_out[:, g * 768:(g + 1) * 768])

        for g in range(G):
            if "g" in parts:
                emit_gather(g)
            if "m" in parts:
                emit_matmuls(g)
            if g >= 2 and "t" in parts:
                emit_tail(g - 2)
        if "t" in parts:
            for g in range(max(G - 2, 0), G):
                emit_tail(g)

    return nc


_NC_CACHE = {}


def _get_nc(profile, nq=NQ, unroll=1, **kw):
    key = (profile, nq, unroll, tuple(sorted(kw.items())))
    if key not in _NC_CACHE:
        nc = _build(profile, unroll=unroll, **kw)
        nc.finalize()
        _NC_CACHE[key] = nc
    return _NC_CACHE[key]


def _prep(inputs):
    tab0 = np.asarray(inputs['tab0'], np.float32)
    tabs = [np.asarray(inputs[f'tab{o+1}'], np.float32) for o in range(3)]
    ids = [np.asarray(inputs[f'ids{o+1}'], np.int64) for o in range(3)]
    cnt = [np.asarray(inputs[f'cnt{o+1}'], np.int64) for o in range(3)]

    # ids/cnt per word with specials folded in: word v<4 -> slot0 points at
    # an appended per-special row, cnt 1
    idsw = []
    cntw = []
    for o in range(3):
        a = ids[o].astype(np.int64).copy()
        c = cnt[o].astype(np.int64).copy()
        a[:4] = 0
        a[:4, 0] = NROWS[o] - 4 + np.arange(4)
        c[:4] = 1
        idsw.append(a)
        cntw.append(c)

    # per-order tables with row0=0, rows 1..V real, +4 special rows
    tabz = []
    for o in range(3):
        nz = NROWS[o]
        tz = np.zeros((nz, EMB), BF16)
        tz[1:nz - 4] = tabs[o][1:].astype(BF16)
        tz[nz - 4:] = tab0[:, o * EMB:(o + 1) * EMB].astype(BF16)
        tabz.append(tz)

    # ---- orders 1/2: global id spaces (<=26+4 and <=676+4 distinct)
    uid = []
    for o in range(2):
        vals = idsw[o].reshape(-1)
        msk = (np.arange(idsw[o].shape[1])[None, :]
               < cntw[o][:, None]).reshape(-1)
        u = np.unique(vals[msk])
        uid.append(u)
    assert len(uid[0]) <= K1 and len(uid[1]) <= K2, \
        (len(uid[0]), len(uid[1]))
    t1r = np.zeros((K1, EMB), BF16)
    t1r[:len(uid[0])] = tabz[0][uid[0]]
    t2r = np.zeros((K2, EMB), BF16)
    t2r[:len(uid[1])] = tabz[1][uid[1]]
    # PE rhs layout: [128, NB2*EMB] with chunk k at cols k*EMB..
    t2r_pack = np.zeros((P, NB2 * EMB), BF16)
    for k in range(NB2):
        t2r_pack[:, k * EMB:(k + 1) * EMB] = t2r[k * P:(k + 1) * P]

    shared = {'t1r': t1r, 't2r': t2r_pack}

    # ---- dedup words, sort by length (cnt1 desc), deal to cores
    x = np.asarray(inputs['x'], np.int64).reshape(-1)
    ux, inv = np.unique(x, return_inverse=True)
    order_u = np.argsort(-cntw[0][ux], kind='stable')
    su = ux[order_u]
    n_u = len(su)
    n_pad = -(-n_u // (NCORES * P)) * (NCORES * P)
    su = np.concatenate([su, np.zeros(n_pad - n_u, np.int64)])
    Gc = n_pad // (NCORES * P)

    core_words = [su[c::NCORES].reshape(Gc, P) for c in range(NCORES)]

    # shared profile: per-group max order-3 cnt across cores
    profile = []
    for g in range(Gc):
        m = 1
        for c in range(NCORES):
            m = max(m, int(cntw[2][core_words[c][g]].max()))
        profile.append((m,))
    profile = tuple(profile)

    # per-word count rows over the order-1/2 id spaces (pure index data)
    def count_matrix(o, K, words_flat):
        nw = len(words_flat)
        lc = np.zeros((nw, K), np.int16)
        idg = idsw[o][words_flat]                      # [nw, L]
        L = idg.shape[1]
        vm = np.arange(L)[None, :] < cntw[o][words_flat][:, None]
        rows = np.repeat(np.arange(nw), L).reshape(nw, L)[vm]
        cols = np.searchsorted(uid[o], idg[vm])
        np.add.at(lc, (rows, cols), 1)
        return lc

    # token -> (core, row) mapping
    rank_of = np.empty(n_u, np.int64)
    rank_of[order_u] = np.arange(n_u)
    tok_rank = rank_of[inv]

    in_maps = []
    for c in range(NCORES):
        words = core_words[c]                          # [Gc, P]
        wflat = words.reshape(-1)

        lc1 = count_matrix(0, K1, wflat)               # [Gc*P, K1]
        lc2 = count_matrix(1, K2, wflat)               # [Gc*P, K2]
        lc1t = np.ascontiguousarray(lc1.T).astype(BF16)    # [K1, Gc*P]
        # lc2 lhsT chunks: [128, NB2*Gc*P], chunk k group g at
        # cols (k*Gc+g)*P ..
        lc2t = np.zeros((P, NB2 * Gc * P), BF16)
        for k in range(NB2):
            blk = lc2[:, k * P:(k + 1) * P].T          # [P, Gc*P]
            lc2t[:, k * Gc * P:(k + 1) * Gc * P] = blk.astype(BF16)

        # ---- order-3 compacted per-core table + idx stream
        key_cols = []
        for g in range(Gc):
            wg = words[g]
            L = profile[g][0]
            idg = idsw[2][wg][:, :L]
            vmask = np.arange(L)[None, :] < cntw[2][wg][:, None]
            keys = np.where(vmask, idg, -1)
            key_cols.append(keys.T)                    # [L, P]
        allk = np.concatenate([k.reshape(-1) for k in key_cols])
        uk = np.unique(allk[allk >= 0])
        assert len(uk) + 1 <= TABC_ROWS, f"{len(uk)=}"
        tabc = np.zeros((TABC_ROWS, EMB), BF16)
        tabc[1:1 + len(uk)] = tabz[2][uk]
        idx_cols = []
        for kcol in key_cols:
            loc = np.where(kcol >= 0,
                           1 + np.searchsorted(uk, np.maximum(kcol, 0)), 0)
            idx_cols.append(loc.astype(np.int16))
        flat = np.concatenate([k.reshape(-1) for k in idx_cols])
        grid16 = flat.reshape(-1, 16).T
        grid = np.tile(grid16, (8, 1))                 # [128, W]

        rcps = []
        for g in range(Gc):
            wg = words[g]
            r = np.stack([1.0 / cntw[o][wg] for o in range(3)], 1)
            rcps.append(r)
        rcp_grid = np.concatenate(rcps, 1).astype(np.float32)

        m = dict(shared)
        m['tabc'] = tabc
        m['idxs'] = np.ascontiguousarray(grid)
        m['rcp'] = np.ascontiguousarray(rcp_grid)
        m['lc1'] = lc1t
        m['lc2'] = np.ascontiguousarray(lc2t)
        in_maps.append(m)
    return in_maps, profile, tok_rank


def _run(nc, in_maps, trace=False):
    return run_bass_kernel_spmd(nc, in_maps, core_ids=list(range(NCORES)),
                                trace=trace)


def kernel(**inputs):
    in_maps, profile, tok_rank = _prep(inputs)
    nc = _get_nc(profile)
    res = _run(nc, in_maps)
    by_rank = np.stack([np.asarray(res.results[c]['out'])
                        for c in range(NCORES)])      # [core, row, 768]
    out = by_rank[tok_rank % NCORES, tok_rank // NCORES]
    return out.reshape(B, S, 768)
